# revision 89
# baseline (speedup 1.0000x reference)
"""Trainium2 Bass kernel for nn_GCNModel (6-layer GCN + 3-layer FC mesh deformer).

Strategy
--------
Data-parallel over batch B=32 across 8 NeuronCores (4 batch elements each).

Algebraic restructuring (host side, exact):
  ReLU only follows GCN layers 2, 4, 6, so each pair of GCN layers collapses:
      A(A x W1 + 1 b1^T) W2 + 1 b2^T
        = A^2 x (W1 W2) + (A 1) (b1 W2)^T + 1 b2^T
  with A the dense-ified normalized adjacency.  Three aggregations with a
  host-precomputed dense A^2 replace six sparse gather/scatter aggregations.
  Pair 1's wide input further reduces to a rank<=6 product folded into a
  single K<=6 matmul (zero-padded to K=128 so the PE activity monitor
  keeps the clock un-throttled).

Device schedule (single pass, PE kept dense the whole way):
  phase0 (A^2 verts, DR fp8)  ->  x1(b0)
  for b: t2(b) -> z2(b) with x1(b+1)/c1(b+1)/t3(b) interleaved into the
         DR matmul stream (no low-activity stretches, no HAM re-throttle)
  pair3 aggregation -> FC head on resident fp8 weights (DoubleRow),
  prefetched over the HWDGE scalar ring during the GCN phase.

Everything runs in bf16/fp8 operands with fp32 PSUM accumulation
(host-validated: ~7e-3 max rel error vs fp32 reference; the output is
dominated by `vertices` plus a 0.1-scaled tanh-squashed deformation).
Output is [BL, V*3] batch-major; the host reshapes (no transposes).
"""

import numpy as np
import ml_dtypes

B, V, E, IMG_F = 32, 2048, 12288, 512
N_CORES = 8
BL = B // N_CORES  # 4 batch elements per core
P = 128
NV = V // P   # 16 vertex chunks
F = 512
NF = F // P   # 4 feature chunks
FC_H = 1024
FLAT = V * 3  # 6144
NV2 = NV // 2  # 8 double-row vertex chunks
G = BL * 3    # 12: (batch, coord) group width

BF16 = ml_dtypes.bfloat16
FP8 = ml_dtypes.float8_e4m3

_CACHE = {}


def _host_prep(inputs):
    """Exact (fp64) host-side algebra: dense A^2, collapsed weights, shards."""
    ei = np.asarray(inputs["edge_index"])
    src = np.concatenate([ei[0], np.arange(V)]).astype(np.int64)
    dst = np.concatenate([ei[1], np.arange(V)]).astype(np.int64)
    deg = np.zeros(V)
    np.add.at(deg, dst, 1.0)
    dinv = 1.0 / np.sqrt(deg)
    normv = dinv[src] * dinv[dst]
    A = np.zeros((V, V))
    np.add.at(A, (dst, src), normv)
    A2 = A @ A
    rho = (A @ np.ones(V)).astype(np.float32)
    rho2 = (A2 @ np.ones(V)).astype(np.float32)

    W = [np.asarray(inputs[f"W{i}"], np.float64) for i in range(1, 7)]
    bb = [np.asarray(inputs[f"b{i}"], np.float64) for i in range(1, 7)]
    W12 = W[0] @ W[1]
    W34 = W[2] @ W[3]
    W56 = W[4] @ W[5]
    bias1 = bb[0] @ W[1]  # pairs with rho
    bias2 = bb[2] @ W[3]
    bias3 = bb[4] @ W[5]
    b2, b4, b6 = bb[1], bb[3], bb[5]

    def pack_rows(w, ncol):
        # [nk*128, ncol] -> [128, nk*ncol] with chunk kc at cols [kc*ncol:...]
        w = np.asarray(w, np.float32)
        nk = w.shape[0] // P
        return np.ascontiguousarray(
            w.reshape(nk, P, ncol).transpose(1, 0, 2).reshape(P, nk * ncol)
        )

    shared = {}
    # A2T in fp8 DoubleRow layout: [uc2][p, j*V + v] = A2T[uc2*256+j*128+p, v]
    A2T = np.ascontiguousarray(A2.T).astype(np.float32)
    shared["A2T"] = np.ascontiguousarray(
        A2T.reshape(NV2, 2, P, V).transpose(0, 2, 1, 3).reshape(NV2, P, 2 * V)
    ).astype(FP8)
    shared["W12A"] = np.asarray(W12[:3], np.float32).astype(BF16)
    bias_pack1 = np.stack([bias1, b2]).astype(np.float32)  # pairs with rho1
    shared["HAS_BIAS1"] = bool(np.any(bias_pack1))
    shared["BIASP1"] = bias_pack1.astype(BF16)
    shared["RHO2"] = rho2.reshape(1, V).astype(BF16)
    shared["RHO1"] = np.stack([rho, np.ones(V, np.float32)]).astype(BF16)
    shared["W12B"] = pack_rows(W12[3:], F).astype(BF16)
    # W34 in fp8 DoubleRow layout: [p, (fc2, j, fout)] = W34[fc2*256+j*128+p, f]
    W34f = np.asarray(W34, np.float32)
    shared["W34"] = np.ascontiguousarray(
        W34f.reshape(2, 2, P, F).transpose(2, 0, 1, 3).reshape(P, 4 * F)
    ).astype(FP8)
    # W56 fp8 DoubleRow, group padded 3->16: [p, q*32+j*16+c] = W56[(2q+j)*128+p, c]
    w56p = np.zeros((P, 2, 2, 16), np.float32)
    w56p[:, :, :, :3] = np.asarray(W56, np.float32).reshape(2, 2, P, 3).transpose(
        2, 0, 1, 3
    )
    shared["W56"] = np.ascontiguousarray(w56p.reshape(P, 64)).astype(FP8)

    bias_pack2 = np.stack([bias2, b4]).astype(np.float32)  # [2, 512]
    # pair3 bias pack in (coord, batch) column order, padded 12->16
    bias_pack3 = np.zeros((2, 16), np.float32)
    for cc in range(3):
        bias_pack3[0, cc * BL:cc * BL + BL] = bias3[cc]
        bias_pack3[1, cc * BL:cc * BL + BL] = b6[cc]
    shared["HAS_BIAS2"] = bool(np.any(bias_pack2))
    shared["HAS_BIAS3"] = bool(np.any(bias_pack3))
    shared["BIASP2"] = bias_pack2.astype(BF16)
    shared["BIASP3"] = bias_pack3.astype(BF16)

    # FC weights, fp8, DoubleRow-packed with K-chunk pairs on the j axis.
    # FC1 K-chunk order m=(cc, u), j: chunk covers flat rows ((2u+j)*128+p)*3+cc
    # matching the (coord-major) x3 stationary layout.
    fcW1 = np.asarray(inputs["fcW1"], np.float32)
    cc_i = np.arange(3)[:, None, None, None]
    u_i = np.arange(NV2)[None, :, None, None]
    j_i = np.arange(2)[None, None, :, None]
    p_i = np.arange(P)[None, None, None, :]
    perm = ((2 * u_i + j_i) * P + p_i) * 3 + cc_i  # [3, 8, 2, 128]
    shared["FCW1"] = np.ascontiguousarray(
        fcW1[perm.reshape(-1)].reshape(3, NV2, 2, P, FC_H)
        .transpose(3, 0, 1, 2, 4).reshape(P, 3 * NV2 * 2 * FC_H)
    ).astype(FP8)
    fcW2 = np.asarray(inputs["fcW2"], np.float32)
    shared["FCW2"] = np.ascontiguousarray(
        fcW2.reshape(4, 2, P, FC_H).transpose(2, 0, 1, 3).reshape(P, 8 * FC_H)
    ).astype(FP8)
    fcW3 = np.asarray(inputs["fcW3"], np.float32)
    shared["FCW3"] = np.ascontiguousarray(
        fcW3.reshape(4, 2, P, FLAT).transpose(2, 0, 1, 3).reshape(P, 8 * FLAT)
    ).astype(FP8)
    fcb1 = np.asarray(inputs["fcb1"], np.float32)
    fcb2 = np.asarray(inputs["fcb2"], np.float32)
    fcb3 = np.asarray(inputs["fcb3"], np.float32)
    shared["HAS_FCB"] = bool(np.any(fcb1) or np.any(fcb2) or np.any(fcb3))
    shared["FCB1"] = np.ascontiguousarray(np.broadcast_to(fcb1, (BL, FC_H)))
    shared["FCB2"] = np.ascontiguousarray(np.broadcast_to(fcb2, (BL, FC_H)))
    shared["FCB3"] = np.ascontiguousarray(np.broadcast_to(fcb3, (BL, FLAT)))

    # per-core shards
    verts = np.asarray(inputs["vertices"], np.float32)  # [B, V, 3]
    img = np.asarray(inputs["img_features"], np.float32)  # [B, 512]
    per_core = []
    for c in range(N_CORES):
        vb = verts[c * BL:(c + 1) * BL]  # [BL, V, 3]
        # DoubleRow lhsT: [p, uc2*32 + j*16 + (b*3+cc)] = verts[b, uc2*256+j*128+p, cc]
        # (group dim padded 12->16: dual-fp8 LDW requires 16B-aligned j-stride)
        vraw = vb.transpose(1, 0, 2).reshape(NV2, 2, P, G)
        vvm = np.zeros((NV2, P, 2, 16), np.float32)
        vvm[:, :, :, :G] = vraw.transpose(0, 2, 1, 3)
        vvm = np.ascontiguousarray(
            vvm.transpose(1, 0, 2, 3).reshape(P, NV2 * 32)
        ).astype(FP8)
        per_core.append({
            "VVM": vvm,
            "IMG": np.ascontiguousarray(img[c * BL:(c + 1) * BL]).astype(BF16),
        })
    return shared, per_core


def _build_program(has_bias1, has_bias2, has_bias3, has_fcb):
    """Emit the Bass/Tile program (identical on all cores)."""
    from concourse import bacc, bass, mybir, tile
    from concourse.masks import make_identity

    f32 = mybir.dt.float32
    bf16 = mybir.dt.bfloat16
    fp8 = mybir.dt.float8e4
    AF = mybir.ActivationFunctionType
    DR = mybir.MatmulPerfMode.DoubleRow

    nc = bacc.Bacc(trn_type="TRN2")

    d_a2t = nc.dram_tensor("A2T", [NV2, P, 2 * V], fp8, kind="ExternalInput")
    d_w12a = nc.dram_tensor("W12A", [3, F], bf16, kind="ExternalInput")
    d_biasp1 = nc.dram_tensor("BIASP1", [2, F], bf16, kind="ExternalInput")
    d_rho2 = nc.dram_tensor("RHO2", [1, V], bf16, kind="ExternalInput")
    d_rho1 = nc.dram_tensor("RHO1", [2, V], bf16, kind="ExternalInput")
    d_w12b = nc.dram_tensor("W12B", [P, 4 * F], bf16, kind="ExternalInput")
    d_w34 = nc.dram_tensor("W34", [P, 4 * F], fp8, kind="ExternalInput")
    d_w56 = nc.dram_tensor("W56", [P, 64], fp8, kind="ExternalInput")
    d_biasp2 = nc.dram_tensor("BIASP2", [2, F], bf16, kind="ExternalInput")
    d_biasp3 = nc.dram_tensor("BIASP3", [2, 16], bf16, kind="ExternalInput")
    d_fcw1 = nc.dram_tensor("FCW1", [P, 3 * NV2 * 2 * FC_H], fp8,
                            kind="ExternalInput")
    d_fcw2 = nc.dram_tensor("FCW2", [P, 8 * FC_H], fp8, kind="ExternalInput")
    d_fcw3 = nc.dram_tensor("FCW3", [P, 8 * FLAT], fp8, kind="ExternalInput")
    d_fcb1 = nc.dram_tensor("FCB1", [BL, FC_H], f32, kind="ExternalInput")
    d_fcb2 = nc.dram_tensor("FCB2", [BL, FC_H], f32, kind="ExternalInput")
    d_fcb3 = nc.dram_tensor("FCB3", [BL, FLAT], f32, kind="ExternalInput")
    d_vvm = nc.dram_tensor("VVM", [P, NV2 * 32], fp8, kind="ExternalInput")
    d_img = nc.dram_tensor("IMG", [BL, IMG_F], bf16, kind="ExternalInput")
    d_out = nc.dram_tensor("OUT", [BL, FLAT], f32, kind="ExternalOutput")

    KX = 6 if has_bias1 else 4  # stat1/rhs1 live rows (c1 + w12a [+ biases])

    with tile.TileContext(nc) as tc:
        with (
            tc.tile_pool(name="const", bufs=1) as const_pool,
            # x1/t2 single-buffered: next batch's writes only start after the
            # previous batch's last PE read in program order, so no stall.
            tc.tile_pool(name="x1p", bufs=1) as x1_pool,
            tc.tile_pool(name="t2p", bufs=1) as t2_pool,
            tc.tile_pool(name="p1", bufs=2) as p1_pool,
            tc.tile_pool(name="work", bufs=2) as work_pool,
            tc.tile_pool(name="t3b", bufs=1) as t3b_pool,
            tc.tile_pool(name="hfin", bufs=1) as hfin_pool,
            tc.tile_pool(name="tail", bufs=2) as tail_pool,
            tc.tile_pool(name="psA", bufs=2, space="PSUM") as psA,
            tc.tile_pool(name="psB", bufs=2, space="PSUM") as psB,
            tc.tile_pool(name="psT", bufs=1, space="PSUM") as psT,
        ):
            # ---------- constant DMA enqueue ----------
            # Critical path first: vvm then the A2T chunks (phase0 consumes
            # them in order), split across the two HWDGE rings.  The small
            # weights follow -- none is needed before ~t=25us.
            vvm = const_pool.tile([P, NV2 * 32], fp8, tag="vvm")
            nc.sync.dma_start(out=vvm[:], in_=d_vvm[:])
            a2t = []
            for uc2 in range(NV2):
                t = const_pool.tile([P, 2 * V], fp8, tag=f"a2t{uc2}")
                a2t.append(t)
            for uc2 in range(NV2):
                eng = nc.sync if uc2 % 2 == 0 else nc.scalar
                eng.dma_start(out=a2t[uc2][:], in_=d_a2t[uc2])
            w12b = const_pool.tile([P, 4 * F], bf16, tag="w12b")
            nc.sync.dma_start(out=w12b[:], in_=d_w12b[:])
            w34 = const_pool.tile([P, 4 * F], fp8, tag="w34")
            nc.sync.dma_start(out=w34[:], in_=d_w34[:])
            w56 = const_pool.tile([P, 64], fp8, tag="w56")
            nc.sync.dma_start(out=w56[:], in_=d_w56[:])
            if has_bias2:
                biasp2 = const_pool.tile([2, F], bf16, tag="biasp2")
                nc.sync.dma_start(out=biasp2[:], in_=d_biasp2[:])
            if has_bias3:
                biasp3 = const_pool.tile([2, 16], bf16, tag="biasp3")
                nc.sync.dma_start(out=biasp3[:], in_=d_biasp3[:])
            if has_bias2 or has_bias1:
                rho1 = const_pool.tile([2, V], bf16, tag="rho1")
                nc.sync.dma_start(out=rho1[:], in_=d_rho1[:])
            # FC weight tiles (fp8, resident); the prefetch DMAs are emitted
            # after x1(0) so the ACT ring only starts them once A2T landed
            # (they'd otherwise steal HBM bandwidth from the critical path).
            fcw1 = const_pool.tile([P, 3 * NV2 * 2 * FC_H], fp8, tag="fcw1")
            fcw2 = const_pool.tile([P, 8 * FC_H], fp8, tag="fcw2")
            fcw3 = const_pool.tile([P, 8 * FLAT], fp8, tag="fcw3")

            ident_bf = const_pool.tile([P, P], bf16, tag="ident_bf")
            make_identity(nc, ident_bf[:])
            scratch = const_pool.tile([1, P], bf16, tag="scratch")
            # HAM keep-alive machinery: full-K dummy matmuls sprinkled into
            # DMA-paced stretches keep the PE activity monitor at K=8/8
            # (2.4 GHz).  Region-scoped psum tile (psT is free outside the
            # z2/t3 loops) with a single reader at close to defeat DCE.

            def ka_open():
                ps_ka = psT.tile([P, 1024], f32, tag="psT")
                return ps_ka

            def ka_mm(ps_ka, n, nn=P):
                # wide keep-alives stream w12b (resident bf16) as rhs
                rhs = ident_bf[:, :nn] if nn <= P else w12b[:, :nn]
                for _ in range(n):
                    nc.tensor.matmul(
                        out=ps_ka[:, :nn],
                        lhsT=ident_bf[:],
                        rhs=rhs,
                        start=True,
                        stop=True,
                    )

            def ka_close(ps_ka):
                nc.vector.tensor_copy(out=scratch[:], in_=ps_ka[:1, :P])

            ka0 = ka_open()
            ka_mm(ka0, 30)

            # feature-major (coord,batch)-ordered t3 rows; pad rows zeroed once
            t3t_all = const_pool.tile([16, V], bf16, tag="t3t_all")
            nc.gpsimd.memset(t3t_all[:], 0.0)
            x2_all = const_pool.tile([P, NF * V], fp8, tag="x2")
            # avt shares its slot with x3t (tag "gvec"): avt is released
            # right after the rhs1 scatter, long before pair3 needs x3t.
            avt_bf = const_pool.tile([G, V], bf16, tag="gvec")

            # ---------- phase 0: A^2 @ verts, feature-major ----------
            # h=0 is paced by the A2T DMA stream (~1.4us/chunk); h=1 runs
            # dense and carries x1(0)'s first-half matmuls inline.
            def phase0_half(h, interleave=None):
                ps = psA.tile([G, 1024], f32, tag="psA")
                for uc2 in range(NV2):
                    lhsT = vvm[:, uc2 * 32:(uc2 + 1) * 32].rearrange(
                        "p (j g) -> p j g", j=2
                    )[:, :, :G]
                    rhs3 = a2t[uc2][:].rearrange("p (j v) -> p j v", j=2)
                    for n2 in range(2):
                        col = h * 1024 + n2 * 512
                        nc.tensor.matmul(
                            out=ps[:, n2 * 512:(n2 + 1) * 512],
                            lhsT=lhsT,
                            rhs=rhs3[:, :, col:col + 512],
                            start=(uc2 == 0),
                            stop=(uc2 == NV2 - 1),
                            perf_mode=DR,
                        )
                    if interleave is not None:
                        interleave(uc2)
                nc.vector.tensor_copy(
                    out=avt_bf[:, h * 1024:(h + 1) * 1024], in_=ps[:]
                )
            def build_p1(b, halves=(0, 1)):
                # pair-1 stationary/stream tiles, zero-padded to K=128 so the
                # PE activity monitor stays engaged.
                # stat1 rows: 0 = img@W12B (on device), 1-3 = W12A rows,
                #             4-5 = bias pack (bias path only), rest zero.
                # rhs1 rows: 0 = rho2, 1-3 = A^2 verts rows, 4-5 = rho1.
                st = p1_pool.tile([P, F], bf16, tag="stat1")
                nc.gpsimd.memset(st[:], 0.0)
                nc.gpsimd.dma_start(out=st[1:4, :], in_=d_w12a[:])
                if has_bias1:
                    nc.gpsimd.dma_start(out=st[4:6, :], in_=d_biasp1[:])
                rh = p1_pool.tile([P, V], bf16, tag="rhs1")
                nc.gpsimd.memset(rh[:], 0.0)
                nc.gpsimd.dma_start(out=rh[0:1, :], in_=d_rho2[:])
                if has_bias1:
                    nc.gpsimd.dma_start(out=rh[4:6, :], in_=d_rho1[:])
                for h in halves:
                    cols = slice(h * 1024, (h + 1) * 1024)
                    nc.gpsimd.dma_start(
                        out=rh[1:4, cols],
                        in_=avt_bf[b * 3:(b + 1) * 3, cols],
                    )
                return st, rh

            def emit_c1(img_sb, st):
                # c1 = img_b @ W12B -> psum [1, F]; then into stat1 row 0
                ps_c1 = psB.tile([1, F], f32, tag="psB")
                for kc in range(4):
                    nc.tensor.matmul(
                        out=ps_c1[:],
                        lhsT=img_sb[:, kc:kc + 1],
                        rhs=w12b[:, kc * F:(kc + 1) * F],
                        start=(kc == 0),
                        stop=(kc == 3),
                    )
                nc.vector.tensor_copy(out=st[0:1, :], in_=ps_c1[:])

            def emit_x1_part(p1, x1_t, step):
                # two of the 16 K=128(zero-padded) pair-1 matmuls + relu->fp8
                st, rh = p1
                for i in range(2):
                    h, rem = divmod(step * 2 + i, 8)
                    fc, n2 = divmod(rem, 2)
                    col = h * 1024 + n2 * 512
                    ps = psB.tile([P, 512], f32, tag="psB")
                    nc.tensor.matmul(
                        out=ps[:],
                        lhsT=st[:, fc * P:(fc + 1) * P],
                        rhs=rh[:, col:col + 512],
                        start=True,
                        stop=True,
                    )
                    if i == 0:
                        nc.vector.tensor_scalar_max(
                            out=x1_t[:, fc * V + col:fc * V + col + 512],
                            in0=ps[:],
                            scalar1=0.0,
                        )
                    else:
                        nc.scalar.activation(
                            out=x1_t[:, fc * V + col:fc * V + col + 512],
                            in_=ps[:],
                            func=AF.Relu,
                        )

            # ---------- phase0 + x1(0) (later batches fold into z2) ------
            img_sb = work_pool.tile([P, 4], bf16, tag="img")
            nc.gpsimd.dma_start(
                out=img_sb[:], in_=d_img[0].rearrange("(c p) -> p c", p=P)
            )
            x1_cur = x1_pool.tile([P, NF * V], fp8, tag="x1")
            phase0_half(0)
            p1_cur = build_p1(0, halves=(0,))
            emit_c1(img_sb, p1_cur[0])

            def ilv0(uc2):
                # x1(0) first-half parts ride inside the dense h=1 stream
                if uc2 in (3, 5, 7):
                    emit_x1_part(p1_cur, x1_cur, (uc2 - 3) // 2)

            phase0_half(1, ilv0)
            # second rhs1 half: avt's h=1 columns only exist now
            nc.gpsimd.dma_start(
                out=p1_cur[1][1:4, 1024:2048], in_=avt_bf[0:3, 1024:2048]
            )
            for step in range(3, 8):
                emit_x1_part(p1_cur, x1_cur, step)
            ka_close(ka0)
            # FC weight prefetch, gated behind the critical-path DMAs: a
            # dummy WAW write into fcw1 (sourced from x1) forces Tile to
            # delay the enqueue until x1(0) exists -- by then A2T has fully
            # landed.  The rings share the 16 SDMA engines, so an early fcw
            # enqueue steals HBM bandwidth from the critical path.
            nc.vector.tensor_copy(out=fcw1[0:1, 0:P], in_=x1_cur[0:1, 0:P])
            nc.vector.tensor_copy(out=fcw2[0:1, 0:P], in_=x1_cur[0:1, 0:P])
            nc.vector.tensor_copy(out=fcw3[0:1, 0:P], in_=x1_cur[0:1, 0:P])
            nc.scalar.dma_start(out=fcw1[:], in_=d_fcw1[:])
            nc.scalar.dma_start(out=fcw2[:], in_=d_fcw2[:])
            nc.scalar.dma_start(out=fcw3[:], in_=d_fcw3[:])

            # ---------- per batch: t2 -> z2 (+ interleaved x1/c1/t3) -------
            w34_3d = w34[:].rearrange("p (k j n) -> p k j n", k=2, j=2)
            w56_3d = w56[:].rearrange("p (q j g) -> p q j g", q=2, j=2)
            t3vm_bf = const_pool.tile([P, NV * 16], bf16, tag="vmbf")
            t3vm_f8 = const_pool.tile([P, NV * 16], fp8, tag="vmf8")
            for b in range(BL):
                # t2 = x1 @ W34, vertex-major fp8, DoubleRow over f
                x1_3d = x1_cur[:].rearrange("p (f v) -> p f v", f=NF)
                t2_f8 = t2_pool.tile([P, NV * F], fp8, tag="t2")
                for vc in range(NV):
                    ps = psB.tile([P, F], f32, tag="psB")
                    for fc2 in range(2):
                        nc.tensor.matmul(
                            out=ps[:],
                            lhsT=x1_3d[:, fc2 * 2:fc2 * 2 + 2,
                                       vc * P:(vc + 1) * P],
                            rhs=w34_3d[:, fc2],
                            start=(fc2 == 0),
                            stop=(fc2 == 1),
                            perf_mode=DR,
                        )
                    if vc % 2 == 0:
                        nc.vector.tensor_copy(
                            out=t2_f8[:, vc * F:(vc + 1) * F], in_=ps[:]
                        )
                    else:
                        nc.scalar.copy(
                            out=t2_f8[:, vc * F:(vc + 1) * F], in_=ps[:]
                        )

                # next batch's stream/stationary prep (DMAs overlap z2)
                if b + 1 < BL:
                    img_sb = work_pool.tile([P, 4], bf16, tag="img")
                    nc.gpsimd.dma_start(
                        out=img_sb[:],
                        in_=d_img[b + 1].rearrange("(c p) -> p c", p=P),
                    )
                    p1_nxt = build_p1(b + 1)
                    x1_nxt = x1_pool.tile([P, NF * V], fp8, tag="x1")

                # z2 = A^2 t2 (feature-major out, DoubleRow); x2 = relu.
                # x1(b+1), c1(b+1) and t3(b) matmuls ride inside this stream
                # so the PE never sees a low-activity window.
                t2_3d = t2_f8[:].rearrange("p (u j f) -> p u j f", u=NV2, j=2)
                x2_3d = x2_all[:].rearrange("p (q j v) -> p q j v", q=2, j=2)
                t3t_rows = t3t_all[:].rearrange("(c b) v -> c b v", b=BL)
                ps_t3 = None
                t3t_b = None
                x1_step = 0
                for fc in range(NF):
                    for nh in range(2):
                        ps = psA.tile([P, 1024], f32, tag="psA")
                        for uc2 in range(NV2):
                            lhsT = t2_3d[:, uc2, :, fc * P:(fc + 1) * P]
                            rhs3 = a2t[uc2][:].rearrange(
                                "p (j v) -> p j v", j=2
                            )
                            for n2 in range(2):
                                col = nh * 1024 + n2 * 512
                                nc.tensor.matmul(
                                    out=ps[:, n2 * 512:(n2 + 1) * 512],
                                    lhsT=lhsT,
                                    rhs=rhs3[:, :, col:col + 512],
                                    start=(uc2 == 0),
                                    stop=(uc2 == NV2 - 1 and not has_bias2),
                                    perf_mode=DR,
                                )
                        if has_bias2:
                            for n2 in range(2):
                                col = nh * 1024 + n2 * 512
                                nc.tensor.matmul(
                                    out=ps[:, n2 * 512:(n2 + 1) * 512],
                                    lhsT=biasp2[:, fc * P:(fc + 1) * P],
                                    rhs=rho1[:, col:col + 512],
                                    start=False,
                                    stop=True,
                                )
                        dst = x2_all[:, fc * V + nh * 1024:
                                     fc * V + (nh + 1) * 1024]
                        if fc == 3:
                            # last groups gate the t3 tail: split across
                            # engines so neither blocks the t3 stores
                            nc.vector.tensor_scalar_max(
                                out=dst[:, 0:512], in0=ps[:, 0:512],
                                scalar1=0.0,
                            )
                            nc.scalar.activation(
                                out=dst[:, 512:1024], in_=ps[:, 512:1024],
                                func=AF.Relu,
                            )
                        elif nh == 0:
                            nc.vector.tensor_scalar_max(
                                out=dst, in0=ps[:], scalar1=0.0
                            )
                        else:
                            nc.scalar.activation(
                                out=dst, in_=ps[:], func=AF.Relu
                            )

                        # interleave: c1 for the next batch in the first slot,
                        # then the 8 x1 parts front-loaded so their relu
                        # copies land before t2(b+1) needs them
                        if fc == 0 and nh == 0 and b + 1 < BL:
                            emit_c1(img_sb, p1_nxt[0])
                        elif b + 1 < BL and x1_step < 8:
                            nparts = 2 if (nh == 0 and x1_step < 6) else 1
                            for _ in range(nparts):
                                if x1_step < 8:
                                    emit_x1_part(p1_nxt, x1_nxt, x1_step)
                                    x1_step += 1
                        # interleave: t3(b) h=0 DoubleRow chunks; q=1 sits
                        # in the (3,0) slot so the h=0 store chain runs
                        # under the final (3,1) group
                        if (fc == 1 and nh == 1) or (fc == 3 and nh == 0):
                            q = fc // 2
                            if q == 0:
                                ps_t3 = psT.tile([16, 1024], f32, tag="psT")
                            for n2 in range(2):
                                nc.tensor.matmul(
                                    out=ps_t3[:, n2 * 512:(n2 + 1) * 512],
                                    lhsT=w56_3d[:, q],
                                    rhs=x2_3d[:, q, :,
                                              n2 * 512:(n2 + 1) * 512],
                                    start=(q == 0),
                                    stop=(q == 1),
                                    perf_mode=DR,
                                )
                            if q == 1:
                                # t3 h=0 store + scatter, under group (3,1)
                                t3t_b = t3b_pool.tile([3, V], bf16,
                                                      tag="t3t_b")
                                nc.vector.tensor_copy(
                                    out=t3t_b[:, 0:512],
                                    in_=ps_t3[0:3, 0:512],
                                )
                                nc.scalar.copy(
                                    out=t3t_b[:, 512:1024],
                                    in_=ps_t3[0:3, 512:1024],
                                )
                                nc.sync.dma_start(
                                    out=t3t_rows[0:3, b, 0:1024],
                                    in_=t3t_b[:, 0:1024],
                                )
                                if b == BL - 1:
                                    nc.sync.dma_start(
                                        out=t3vm_bf[:, 0:128].rearrange(
                                            "p (v g) -> p v g", g=16),
                                        in_=t3t_all[0:16, 0:1024],
                                        transpose=True,
                                    )
                                    nc.vector.tensor_copy(
                                        out=t3vm_f8[:, 0:128],
                                        in_=t3vm_bf[:, 0:128],
                                    )

                # any leftover interleave steps for the next batch's x1
                while b + 1 < BL and x1_step < 8:
                    emit_x1_part(p1_nxt, x1_nxt, x1_step)
                    x1_step += 1

                # t3(b) h=1 compute + store (short tail, then t2(b+1))
                ps_t3 = psT.tile([16, 1024], f32, tag="psT")
                for q in range(2):
                    for n2 in range(2):
                        col = 1024 + n2 * 512
                        nc.tensor.matmul(
                            out=ps_t3[:, n2 * 512:(n2 + 1) * 512],
                            lhsT=w56_3d[:, q],
                            rhs=x2_3d[:, q, :, col:col + 512],
                            start=(q == 0),
                            stop=(q == 1),
                            perf_mode=DR,
                        )
                nc.vector.tensor_copy(
                    out=t3t_b[:, 1024:1536], in_=ps_t3[0:3, 0:512]
                )
                nc.scalar.copy(
                    out=t3t_b[:, 1536:2048], in_=ps_t3[0:3, 512:1024]
                )
                # h=1 store chain rides the SCALAR ring: the scheduler packs
                # all regular DMAs of a ring ahead of its XBAR transposes,
                # so keeping this off the sync ring lets the (critical)
                # h=0 transpose start without waiting for the h=1 copies
                nc.scalar.dma_start(
                    out=t3t_rows[0:3, b, 1024:2048], in_=t3t_b[:, 1024:2048]
                )
                if b == BL - 1:
                    nc.scalar.dma_start(
                        out=t3vm_bf[:, 128:256].rearrange(
                            "p (v g) -> p v g", g=16),
                        in_=t3t_all[0:16, 1024:2048],
                        transpose=True,
                    )
                    nc.vector.tensor_copy(
                        out=t3vm_f8[:, 128:256], in_=t3vm_bf[:, 128:256]
                    )
                if b + 1 < BL:
                    x1_cur = x1_nxt
                    p1_cur = p1_nxt

            # ---------- pair3 aggregation ----------
            # u-chunks 0-3 need only the early-transposed first t3 half, so
            # they run right after batch 3's tail; chunks 4-7 follow once
            # the second half's transpose (issued mid-tail) lands.  Each
            # output half's relu/transpose/cast is emitted before the other
            # half's remaining matmuls to hide the x3 XBAR latency.
            t3vm_3d = t3vm_f8[:].rearrange("p (u j g) -> p u j g", u=NV2, j=2)
            x3t = const_pool.tile([16, V], bf16, tag="gvec")
            x3vm_bf = const_pool.tile([P, NV * 16], bf16, tag="vmbf2")
            x3f8 = const_pool.tile([P, NV * 16], fp8, tag="vmf82")
            ps_agg = []
            for h in range(2):
                ps_a = psA.tile([16, 1024], f32, tag="psA")
                ps_agg.append(ps_a)

            def agg_part(h, uc2_range, close):
                ps = ps_agg[h]
                for uc2 in uc2_range:
                    rhs3 = a2t[uc2][:].rearrange("p (j v) -> p j v", j=2)
                    for n2 in range(2):
                        col = h * 1024 + n2 * 512
                        nc.tensor.matmul(
                            out=ps[:, n2 * 512:(n2 + 1) * 512],
                            lhsT=t3vm_3d[:, uc2],
                            rhs=rhs3[:, :, col:col + 512],
                            start=(uc2 == 0),
                            stop=(uc2 == NV2 - 1 and close
                                  and not has_bias3),
                            perf_mode=DR,
                        )
                if not close:
                    return
                if has_bias3:
                    for n2 in range(2):
                        col = h * 1024 + n2 * 512
                        nc.tensor.matmul(
                            out=ps[:, n2 * 512:(n2 + 1) * 512],
                            lhsT=biasp3[:],
                            rhs=rho1[:, col:col + 512],
                            start=False,
                            stop=True,
                        )
                nc.vector.tensor_scalar_max(
                    out=x3t[:, h * 1024:(h + 1) * 1024],
                    in0=ps[0:16, :],
                    scalar1=0.0,
                )
                nc.sync.dma_start(
                    out=x3vm_bf[:, h * 128:(h + 1) * 128].rearrange(
                        "p (v g) -> p v g", g=16),
                    in_=x3t[0:16, h * 1024:(h + 1) * 1024],
                    transpose=True,
                )
                nc.vector.tensor_copy(
                    out=x3f8[:, h * 128:(h + 1) * 128],
                    in_=x3vm_bf[:, h * 128:(h + 1) * 128],
                )

            agg_part(0, range(4), False)
            agg_part(1, range(4), False)
            agg_part(0, range(4, NV2), True)
            agg_part(1, range(4, NV2), True)

            # ---------- FC head: resident fp8 weights, DoubleRow, M=BL -----
            # Each hidden layer's transpose runs per n2-half: the first half
            # transposes (XBAR DMA) under the second half's matmuls, and the
            # next layer's first two DoubleRow K-chunks only need the first
            # half, so almost no transpose latency is exposed.
            ka4 = ka_open()

            def h_to_T(ps_h, fcb_dram, emit_group, tg):
                # independent tags: layer N+1's first cast must not wait on
                # layer N's tile release mid-pipeline
                h_sb = hfin_pool.tile([16, FC_H], bf16, tag="hfin" + tg)
                hT_bf = const_pool.tile([P, 8 * 16], bf16, tag="hTbf" + tg)
                hT = const_pool.tile([P, 8 * 16], fp8, tag="hT" + tg)
                if has_fcb:
                    fcb_sb = tail_pool.tile([BL, FC_H], f32, tag="fcb")
                    nc.sync.dma_start(out=fcb_sb[:], in_=fcb_dram[:])
                for n2 in range(2):
                    emit_group(n2)
                    half = slice(n2 * 512, (n2 + 1) * 512)
                    if has_fcb:
                        nc.vector.tensor_add(
                            out=h_sb[0:BL, half], in0=ps_h[:, half],
                            in1=fcb_sb[:, half],
                        )
                    else:
                        # partition-thin copy: split across both engines
                        q0 = n2 * 512
                        nc.vector.tensor_copy(
                            out=h_sb[0:BL, q0:q0 + 256],
                            in_=ps_h[:, q0:q0 + 256],
                        )
                        nc.scalar.copy(
                            out=h_sb[0:BL, q0 + 256:q0 + 512],
                            in_=ps_h[:, q0 + 256:q0 + 512],
                        )
                    nc.sync.dma_start(
                        out=hT_bf[:, n2 * 64:(n2 + 1) * 64].rearrange(
                            "p (k g) -> p k g", g=16),
                        in_=h_sb[0:16, half],
                        transpose=True,
                    )
                    nc.vector.tensor_copy(
                        out=hT[:, n2 * 64:(n2 + 1) * 64],
                        in_=hT_bf[:, n2 * 64:(n2 + 1) * 64],
                    )
                return hT[:].rearrange("p (q j g) -> p q j g", q=4, j=2)

            x3_3d = x3f8[:].rearrange("p (u j g) -> p u j g", u=NV2, j=2)
            fcw1_3d = fcw1[:].rearrange("p (m j n) -> p m j n", m=24, j=2)
            ps_h1 = psA.tile([BL, FC_H], f32, tag="psA")

            def fc1_group(n2):
                # u-major: the first 12 chunks only need the first x3 half,
                # whose transpose completed during the h=1 aggregation
                for i, (u, cc) in enumerate(
                    (u, cc) for u in range(NV2) for cc in range(3)
                ):
                    nc.tensor.matmul(
                        out=ps_h1[:, n2 * 512:(n2 + 1) * 512],
                        lhsT=x3_3d[:, u, :, cc * BL:(cc + 1) * BL],
                        rhs=fcw1_3d[:, cc * NV2 + u, :,
                                    n2 * 512:(n2 + 1) * 512],
                        start=(i == 0),
                        stop=(i == 23),
                        perf_mode=DR,
                    )

            h1T_3d = h_to_T(ps_h1, d_fcb1, fc1_group, "1")
            ka_mm(ka4, 1)

            fcw2_3d = fcw2[:].rearrange("p (q j n) -> p q j n", q=4, j=2)
            ps_h2 = psA.tile([BL, FC_H], f32, tag="psA")

            def fc2_group(n2):
                for q in range(4):
                    nc.tensor.matmul(
                        out=ps_h2[:, n2 * 512:(n2 + 1) * 512],
                        lhsT=h1T_3d[:, q, :, 0:BL],
                        rhs=fcw2_3d[:, q, :, n2 * 512:(n2 + 1) * 512],
                        start=(q == 0),
                        stop=(q == 3),
                        perf_mode=DR,
                    )

            h2T_3d = h_to_T(ps_h2, d_fcb2, fc2_group, "2")
            ka_mm(ka4, 1)
            ka_close(ka4)

            # FC3 + tanh tail (scale by 0.1 and the vertices add happen on
            # the host -- [BL, 1024] tiles are partition-thin and slow on DVE)
            fcw3_3d = fcw3[:].rearrange("p (q j n) -> p q j n", q=4, j=2)
            for ch in range(FLAT // FC_H):
                ps = psA.tile([BL, FC_H], f32, tag="psA")
                for n2 in range(2):
                    for q in range(4):
                        nc.tensor.matmul(
                            out=ps[:, n2 * 512:(n2 + 1) * 512],
                            lhsT=h2T_3d[:, q, :, 0:BL],
                            rhs=fcw3_3d[:, q, :,
                                        ch * FC_H + n2 * 512:
                                        ch * FC_H + (n2 + 1) * 512],
                            start=(q == 0),
                            stop=(q == 3),
                            perf_mode=DR,
                        )
                h3 = tail_pool.tile([BL, FC_H], f32, tag="h3")
                if has_fcb:
                    fcb3_sb = tail_pool.tile([BL, FC_H], f32, tag="fcb")
                    nc.gpsimd.dma_start(
                        out=fcb3_sb[:],
                        in_=d_fcb3[:, ch * FC_H:(ch + 1) * FC_H],
                    )
                    nc.vector.tensor_add(out=h3[:], in0=ps[:], in1=fcb3_sb[:])
                    nc.scalar.activation(out=h3[:], in_=h3[:], func=AF.Tanh)
                else:
                    nc.scalar.activation(out=h3[:], in_=ps[:], func=AF.Tanh)
                nc.sync.dma_start(
                    out=d_out[:, ch * FC_H:(ch + 1) * FC_H], in_=h3[:]
                )

    nc.finalize()
    return nc


def build_in_maps(inputs):
    """Host prep + per-core input maps (exposed for testing)."""
    shared, per_core = _host_prep(inputs)
    key = (shared["HAS_BIAS1"], shared["HAS_BIAS2"], shared["HAS_BIAS3"],
           shared["HAS_FCB"])
    shared_arrays = {k: v for k, v in shared.items() if isinstance(v, np.ndarray)}
    in_maps = []
    for c in range(N_CORES):
        m = dict(shared_arrays)
        m.update(per_core[c])
        in_maps.append(m)
    return key, in_maps


def unpack_out(raw, verts_shard):
    # device returns tanh(h3); deformation scale + vertex add on host
    d = np.asarray(raw, np.float32).reshape(BL, V, 3)
    return np.asarray(verts_shard, np.float32) + 0.1 * d


def kernel(**inputs):
    key, in_maps = build_in_maps(inputs)
    if key not in _CACHE:
        _CACHE[key] = _build_program(*key)
    nc = _CACHE[key]

    from concourse.bass_utils import run_bass_kernel_spmd

    res = run_bass_kernel_spmd(nc, in_maps, list(range(N_CORES)))
    verts = np.asarray(inputs["vertices"], np.float32)
    out = np.empty((B, V, 3), np.float32)
    for c in range(N_CORES):
        out[c * BL:(c + 1) * BL] = unpack_out(
            res.results[c]["OUT"], verts[c * BL:(c + 1) * BL]
        )
    return out


# revision 93
# speedup vs baseline: 1.0226x; 1.0226x over previous
"""Trainium2 Bass kernel for nn_GCNModel (6-layer GCN + 3-layer FC mesh deformer).

Strategy
--------
Data-parallel over batch B=32 across 8 NeuronCores (4 batch elements each).

Algebraic restructuring (host side, exact):
  ReLU only follows GCN layers 2, 4, 6, so each pair of GCN layers collapses:
      A(A x W1 + 1 b1^T) W2 + 1 b2^T
        = A^2 x (W1 W2) + (A 1) (b1 W2)^T + 1 b2^T
  with A the dense-ified normalized adjacency.  Three aggregations with a
  host-precomputed dense A^2 replace six sparse gather/scatter aggregations.
  Pair 1's wide input further reduces to a rank<=6 product folded into a
  single K<=6 matmul (zero-padded to K=128 so the PE activity monitor
  keeps the clock un-throttled).

Device schedule (single pass, PE kept dense the whole way):
  phase0 (A^2 verts, DR fp8)  ->  x1(b0)
  for b: t2(b) -> z2(b) with x1(b+1)/c1(b+1)/t3(b) interleaved into the
         DR matmul stream (no low-activity stretches, no HAM re-throttle)
  pair3 aggregation -> FC head on resident fp8 weights (DoubleRow),
  prefetched over the HWDGE scalar ring during the GCN phase.

Everything runs in bf16/fp8 operands with fp32 PSUM accumulation
(host-validated: ~7e-3 max rel error vs fp32 reference; the output is
dominated by `vertices` plus a 0.1-scaled tanh-squashed deformation).
Output is [BL, V*3] batch-major; the host reshapes (no transposes).
"""

import numpy as np
import ml_dtypes

B, V, E, IMG_F = 32, 2048, 12288, 512
N_CORES = 8
BL = B // N_CORES  # 4 batch elements per core
P = 128
NV = V // P   # 16 vertex chunks
F = 512
NF = F // P   # 4 feature chunks
FC_H = 1024
FLAT = V * 3  # 6144
NV2 = NV // 2  # 8 double-row vertex chunks
G = BL * 3    # 12: (batch, coord) group width

BF16 = ml_dtypes.bfloat16
FP8 = ml_dtypes.float8_e4m3

_CACHE = {}


def _host_prep(inputs):
    """Exact (fp64) host-side algebra: dense A^2, collapsed weights, shards."""
    ei = np.asarray(inputs["edge_index"])
    src = np.concatenate([ei[0], np.arange(V)]).astype(np.int64)
    dst = np.concatenate([ei[1], np.arange(V)]).astype(np.int64)
    deg = np.zeros(V)
    np.add.at(deg, dst, 1.0)
    dinv = 1.0 / np.sqrt(deg)
    normv = dinv[src] * dinv[dst]
    A = np.zeros((V, V))
    np.add.at(A, (dst, src), normv)
    A2 = A @ A
    rho = (A @ np.ones(V)).astype(np.float32)
    rho2 = (A2 @ np.ones(V)).astype(np.float32)

    W = [np.asarray(inputs[f"W{i}"], np.float64) for i in range(1, 7)]
    bb = [np.asarray(inputs[f"b{i}"], np.float64) for i in range(1, 7)]
    W12 = W[0] @ W[1]
    W34 = W[2] @ W[3]
    W56 = W[4] @ W[5]
    bias1 = bb[0] @ W[1]  # pairs with rho
    bias2 = bb[2] @ W[3]
    bias3 = bb[4] @ W[5]
    b2, b4, b6 = bb[1], bb[3], bb[5]

    def pack_rows(w, ncol):
        # [nk*128, ncol] -> [128, nk*ncol] with chunk kc at cols [kc*ncol:...]
        w = np.asarray(w, np.float32)
        nk = w.shape[0] // P
        return np.ascontiguousarray(
            w.reshape(nk, P, ncol).transpose(1, 0, 2).reshape(P, nk * ncol)
        )

    shared = {}
    # A2T in fp8 DoubleRow layout: [uc2][p, j*V + v] = A2T[uc2*256+j*128+p, v]
    A2T = np.ascontiguousarray(A2.T).astype(np.float32)
    shared["A2T"] = np.ascontiguousarray(
        A2T.reshape(NV2, 2, P, V).transpose(0, 2, 1, 3).reshape(NV2, P, 2 * V)
    ).astype(FP8)
    shared["W12A"] = np.asarray(W12[:3], np.float32).astype(BF16)
    bias_pack1 = np.stack([bias1, b2]).astype(np.float32)  # pairs with rho1
    shared["HAS_BIAS1"] = bool(np.any(bias_pack1))
    shared["BIASP1"] = bias_pack1.astype(BF16)
    shared["RHO2"] = rho2.reshape(1, V).astype(BF16)
    shared["RHO1"] = np.stack([rho, np.ones(V, np.float32)]).astype(BF16)
    shared["W12B"] = pack_rows(W12[3:], F).astype(BF16)
    # W34 in fp8 DoubleRow layout: [p, (fc2, j, fout)] = W34[fc2*256+j*128+p, f]
    W34f = np.asarray(W34, np.float32)
    shared["W34"] = np.ascontiguousarray(
        W34f.reshape(2, 2, P, F).transpose(2, 0, 1, 3).reshape(P, 4 * F)
    ).astype(FP8)
    # W56 fp8 DoubleRow, group padded 3->16: [p, q*32+j*16+c] = W56[(2q+j)*128+p, c]
    w56p = np.zeros((P, 2, 2, 16), np.float32)
    w56p[:, :, :, :3] = np.asarray(W56, np.float32).reshape(2, 2, P, 3).transpose(
        2, 0, 1, 3
    )
    shared["W56"] = np.ascontiguousarray(w56p.reshape(P, 64)).astype(FP8)

    bias_pack2 = np.stack([bias2, b4]).astype(np.float32)  # [2, 512]
    # pair3 bias pack in (coord, batch) column order, padded 12->16
    bias_pack3 = np.zeros((2, 16), np.float32)
    for cc in range(3):
        bias_pack3[0, cc * BL:cc * BL + BL] = bias3[cc]
        bias_pack3[1, cc * BL:cc * BL + BL] = b6[cc]
    shared["HAS_BIAS2"] = bool(np.any(bias_pack2))
    shared["HAS_BIAS3"] = bool(np.any(bias_pack3))
    shared["BIASP2"] = bias_pack2.astype(BF16)
    shared["BIASP3"] = bias_pack3.astype(BF16)

    # FC weights, fp8, DoubleRow-packed with K-chunk pairs on the j axis.
    # FC1 K-chunk order m=(cc, u), j: chunk covers flat rows ((2u+j)*128+p)*3+cc
    # matching the (coord-major) x3 stationary layout.
    fcW1 = np.asarray(inputs["fcW1"], np.float32)
    cc_i = np.arange(3)[:, None, None, None]
    u_i = np.arange(NV2)[None, :, None, None]
    j_i = np.arange(2)[None, None, :, None]
    p_i = np.arange(P)[None, None, None, :]
    perm = ((2 * u_i + j_i) * P + p_i) * 3 + cc_i  # [3, 8, 2, 128]
    shared["FCW1"] = np.ascontiguousarray(
        fcW1[perm.reshape(-1)].reshape(3, NV2, 2, P, FC_H)
        .transpose(3, 0, 1, 2, 4).reshape(P, 3 * NV2 * 2 * FC_H)
    ).astype(FP8)
    fcW2 = np.asarray(inputs["fcW2"], np.float32)
    shared["FCW2"] = np.ascontiguousarray(
        fcW2.reshape(4, 2, P, FC_H).transpose(2, 0, 1, 3).reshape(P, 8 * FC_H)
    ).astype(FP8)
    fcW3 = np.asarray(inputs["fcW3"], np.float32)
    shared["FCW3"] = np.ascontiguousarray(
        fcW3.reshape(4, 2, P, FLAT).transpose(2, 0, 1, 3).reshape(P, 8 * FLAT)
    ).astype(FP8)
    fcb1 = np.asarray(inputs["fcb1"], np.float32)
    fcb2 = np.asarray(inputs["fcb2"], np.float32)
    fcb3 = np.asarray(inputs["fcb3"], np.float32)
    shared["HAS_FCB"] = bool(np.any(fcb1) or np.any(fcb2) or np.any(fcb3))
    shared["FCB1"] = np.ascontiguousarray(np.broadcast_to(fcb1, (BL, FC_H)))
    shared["FCB2"] = np.ascontiguousarray(np.broadcast_to(fcb2, (BL, FC_H)))
    shared["FCB3"] = np.ascontiguousarray(np.broadcast_to(fcb3, (BL, FLAT)))

    # per-core shards
    verts = np.asarray(inputs["vertices"], np.float32)  # [B, V, 3]
    img = np.asarray(inputs["img_features"], np.float32)  # [B, 512]
    per_core = []
    for c in range(N_CORES):
        vb = verts[c * BL:(c + 1) * BL]  # [BL, V, 3]
        # DoubleRow lhsT: [p, uc2*32 + j*16 + (b*3+cc)] = verts[b, uc2*256+j*128+p, cc]
        # (group dim padded 12->16: dual-fp8 LDW requires 16B-aligned j-stride)
        vraw = vb.transpose(1, 0, 2).reshape(NV2, 2, P, G)
        vvm = np.zeros((NV2, P, 2, 16), np.float32)
        vvm[:, :, :, :G] = vraw.transpose(0, 2, 1, 3)
        vvm = np.ascontiguousarray(
            vvm.transpose(1, 0, 2, 3).reshape(P, NV2 * 32)
        ).astype(FP8)
        per_core.append({
            "VVM": vvm,
            "IMG": np.ascontiguousarray(img[c * BL:(c + 1) * BL]).astype(BF16),
        })
    return shared, per_core


def _build_program(has_bias1, has_bias2, has_bias3, has_fcb):
    """Emit the Bass/Tile program (identical on all cores)."""
    from concourse import bacc, bass, mybir, tile
    from concourse.masks import make_identity

    f32 = mybir.dt.float32
    bf16 = mybir.dt.bfloat16
    fp8 = mybir.dt.float8e4
    AF = mybir.ActivationFunctionType
    DR = mybir.MatmulPerfMode.DoubleRow

    nc = bacc.Bacc(trn_type="TRN2")

    d_a2t = nc.dram_tensor("A2T", [NV2, P, 2 * V], fp8, kind="ExternalInput")
    d_w12a = nc.dram_tensor("W12A", [3, F], bf16, kind="ExternalInput")
    d_biasp1 = nc.dram_tensor("BIASP1", [2, F], bf16, kind="ExternalInput")
    d_rho2 = nc.dram_tensor("RHO2", [1, V], bf16, kind="ExternalInput")
    d_rho1 = nc.dram_tensor("RHO1", [2, V], bf16, kind="ExternalInput")
    d_w12b = nc.dram_tensor("W12B", [P, 4 * F], bf16, kind="ExternalInput")
    d_w34 = nc.dram_tensor("W34", [P, 4 * F], fp8, kind="ExternalInput")
    d_w56 = nc.dram_tensor("W56", [P, 64], fp8, kind="ExternalInput")
    d_biasp2 = nc.dram_tensor("BIASP2", [2, F], bf16, kind="ExternalInput")
    d_biasp3 = nc.dram_tensor("BIASP3", [2, 16], bf16, kind="ExternalInput")
    d_fcw1 = nc.dram_tensor("FCW1", [P, 3 * NV2 * 2 * FC_H], fp8,
                            kind="ExternalInput")
    d_fcw2 = nc.dram_tensor("FCW2", [P, 8 * FC_H], fp8, kind="ExternalInput")
    d_fcw3 = nc.dram_tensor("FCW3", [P, 8 * FLAT], fp8, kind="ExternalInput")
    d_fcb1 = nc.dram_tensor("FCB1", [BL, FC_H], f32, kind="ExternalInput")
    d_fcb2 = nc.dram_tensor("FCB2", [BL, FC_H], f32, kind="ExternalInput")
    d_fcb3 = nc.dram_tensor("FCB3", [BL, FLAT], f32, kind="ExternalInput")
    d_vvm = nc.dram_tensor("VVM", [P, NV2 * 32], fp8, kind="ExternalInput")
    d_img = nc.dram_tensor("IMG", [BL, IMG_F], bf16, kind="ExternalInput")
    d_out = nc.dram_tensor("OUT", [BL, FLAT], f32, kind="ExternalOutput")

    KX = 6 if has_bias1 else 4  # stat1/rhs1 live rows (c1 + w12a [+ biases])

    with tile.TileContext(nc) as tc:
        with (
            tc.tile_pool(name="const", bufs=1) as const_pool,
            # x1/t2 single-buffered: next batch's writes only start after the
            # previous batch's last PE read in program order, so no stall.
            tc.tile_pool(name="x1p", bufs=1) as x1_pool,
            tc.tile_pool(name="t2p", bufs=1) as t2_pool,
            tc.tile_pool(name="p1", bufs=2) as p1_pool,
            tc.tile_pool(name="work", bufs=2) as work_pool,
            tc.tile_pool(name="t3b", bufs=1) as t3b_pool,
            tc.tile_pool(name="hfin", bufs=1) as hfin_pool,
            tc.tile_pool(name="tail", bufs=2) as tail_pool,
            tc.tile_pool(name="psA", bufs=2, space="PSUM") as psA,
            tc.tile_pool(name="psB", bufs=2, space="PSUM") as psB,
            tc.tile_pool(name="psT", bufs=1, space="PSUM") as psT,
        ):
            # ---------- constant DMA enqueue ----------
            # Critical path first: vvm then the A2T chunks (phase0 consumes
            # them in order), split across the two HWDGE rings.  The small
            # weights follow -- none is needed before ~t=25us.
            vvm = const_pool.tile([P, NV2 * 32], fp8, tag="vvm")
            nc.sync.dma_start(out=vvm[:], in_=d_vvm[:])
            a2t = []
            for uc2 in range(NV2):
                t = const_pool.tile([P, 2 * V], fp8, tag=f"a2t{uc2}")
                a2t.append(t)
            for uc2 in range(NV2):
                eng = nc.sync if uc2 % 2 == 0 else nc.scalar
                eng.dma_start(out=a2t[uc2][:], in_=d_a2t[uc2])
            w12b = const_pool.tile([P, 4 * F], bf16, tag="w12b")
            nc.sync.dma_start(out=w12b[:], in_=d_w12b[:])
            w34 = const_pool.tile([P, 4 * F], fp8, tag="w34")
            nc.sync.dma_start(out=w34[:], in_=d_w34[:])
            w56 = const_pool.tile([P, 64], fp8, tag="w56")
            nc.sync.dma_start(out=w56[:], in_=d_w56[:])
            if has_bias2:
                biasp2 = const_pool.tile([2, F], bf16, tag="biasp2")
                nc.sync.dma_start(out=biasp2[:], in_=d_biasp2[:])
            if has_bias3:
                biasp3 = const_pool.tile([2, 16], bf16, tag="biasp3")
                nc.sync.dma_start(out=biasp3[:], in_=d_biasp3[:])
            if has_bias2 or has_bias1:
                rho1 = const_pool.tile([2, V], bf16, tag="rho1")
                nc.sync.dma_start(out=rho1[:], in_=d_rho1[:])
            # FC weight tiles (fp8, resident); the prefetch DMAs are emitted
            # after x1(0) so the ACT ring only starts them once A2T landed
            # (they'd otherwise steal HBM bandwidth from the critical path).
            fcw1 = const_pool.tile([P, 3 * NV2 * 2 * FC_H], fp8, tag="fcw1")
            fcw2 = const_pool.tile([P, 8 * FC_H], fp8, tag="fcw2")
            fcw3 = const_pool.tile([P, 8 * FLAT], fp8, tag="fcw3")

            ident_bf = const_pool.tile([P, P], bf16, tag="ident_bf")
            make_identity(nc, ident_bf[:])
            scratch = const_pool.tile([1, P], bf16, tag="scratch")
            # HAM keep-alive machinery: full-K dummy matmuls sprinkled into
            # DMA-paced stretches keep the PE activity monitor at K=8/8
            # (2.4 GHz).  Region-scoped psum tile (psT is free outside the
            # z2/t3 loops) with a single reader at close to defeat DCE.

            def ka_open():
                ps_ka = psT.tile([P, 1024], f32, tag="psT")
                return ps_ka

            def ka_mm(ps_ka, n, nn=P):
                # wide keep-alives stream w12b (resident bf16) as rhs
                rhs = ident_bf[:, :nn] if nn <= P else w12b[:, :nn]
                for _ in range(n):
                    nc.tensor.matmul(
                        out=ps_ka[:, :nn],
                        lhsT=ident_bf[:],
                        rhs=rhs,
                        start=True,
                        stop=True,
                    )

            def ka_close(ps_ka):
                nc.vector.tensor_copy(out=scratch[:], in_=ps_ka[:1, :P])

            ka0 = ka_open()
            ka_mm(ka0, 30)

            # feature-major (coord,batch)-ordered t3 rows; pad rows zeroed once
            t3t_all = const_pool.tile([16, V], bf16, tag="t3t_all")
            nc.gpsimd.memset(t3t_all[:], 0.0)
            x2_all = const_pool.tile([P, NF * V], fp8, tag="x2")
            # avt shares its slot with x3t (tag "gvec"): avt is released
            # right after the rhs1 scatter, long before pair3 needs x3t.
            avt_bf = const_pool.tile([G, V], bf16, tag="gvec")

            # ---------- phase 0: A^2 @ verts, feature-major ----------
            # h=0 is paced by the A2T DMA stream (~1.4us/chunk); h=1 runs
            # dense and carries x1(0)'s first-half matmuls inline.
            def phase0_half(h, interleave=None):
                ps = psA.tile([G, 1024], f32, tag="psA")
                for uc2 in range(NV2):
                    lhsT = vvm[:, uc2 * 32:(uc2 + 1) * 32].rearrange(
                        "p (j g) -> p j g", j=2
                    )[:, :, :G]
                    rhs3 = a2t[uc2][:].rearrange("p (j v) -> p j v", j=2)
                    for n2 in range(2):
                        col = h * 1024 + n2 * 512
                        nc.tensor.matmul(
                            out=ps[:, n2 * 512:(n2 + 1) * 512],
                            lhsT=lhsT,
                            rhs=rhs3[:, :, col:col + 512],
                            start=(uc2 == 0),
                            stop=(uc2 == NV2 - 1),
                            perf_mode=DR,
                        )
                    if interleave is not None:
                        interleave(uc2)
                nc.vector.tensor_copy(
                    out=avt_bf[:, h * 1024:(h + 1) * 1024], in_=ps[:]
                )
            def build_p1(b, halves=(0, 1)):
                # pair-1 stationary/stream tiles, zero-padded to K=128 so the
                # PE activity monitor stays engaged.
                # stat1 rows: 0 = img@W12B (on device), 1-3 = W12A rows,
                #             4-5 = bias pack (bias path only), rest zero.
                # rhs1 rows: 0 = rho2, 1-3 = A^2 verts rows, 4-5 = rho1.
                st = p1_pool.tile([P, F], bf16, tag="stat1")
                nc.gpsimd.memset(st[:], 0.0)
                nc.gpsimd.dma_start(out=st[1:4, :], in_=d_w12a[:])
                if has_bias1:
                    nc.gpsimd.dma_start(out=st[4:6, :], in_=d_biasp1[:])
                rh = p1_pool.tile([P, V], bf16, tag="rhs1")
                nc.gpsimd.memset(rh[:], 0.0)
                nc.gpsimd.dma_start(out=rh[0:1, :], in_=d_rho2[:])
                if has_bias1:
                    nc.gpsimd.dma_start(out=rh[4:6, :], in_=d_rho1[:])
                for h in halves:
                    cols = slice(h * 1024, (h + 1) * 1024)
                    nc.gpsimd.dma_start(
                        out=rh[1:4, cols],
                        in_=avt_bf[b * 3:(b + 1) * 3, cols],
                    )
                return st, rh

            def emit_c1(img_sb, st):
                # c1 = img_b @ W12B -> psum [1, F]; then into stat1 row 0
                ps_c1 = psB.tile([1, F], f32, tag="psB")
                for kc in range(4):
                    nc.tensor.matmul(
                        out=ps_c1[:],
                        lhsT=img_sb[:, kc:kc + 1],
                        rhs=w12b[:, kc * F:(kc + 1) * F],
                        start=(kc == 0),
                        stop=(kc == 3),
                    )
                nc.vector.tensor_copy(out=st[0:1, :], in_=ps_c1[:])

            def emit_x1_part(p1, x1_t, step):
                # two of the 16 K=128(zero-padded) pair-1 matmuls + relu->fp8
                st, rh = p1
                for i in range(2):
                    h, rem = divmod(step * 2 + i, 8)
                    fc, n2 = divmod(rem, 2)
                    col = h * 1024 + n2 * 512
                    ps = psB.tile([P, 512], f32, tag="psB")
                    nc.tensor.matmul(
                        out=ps[:],
                        lhsT=st[:, fc * P:(fc + 1) * P],
                        rhs=rh[:, col:col + 512],
                        start=True,
                        stop=True,
                    )
                    if i == 0:
                        nc.vector.tensor_scalar_max(
                            out=x1_t[:, fc * V + col:fc * V + col + 512],
                            in0=ps[:],
                            scalar1=0.0,
                        )
                    else:
                        nc.scalar.activation(
                            out=x1_t[:, fc * V + col:fc * V + col + 512],
                            in_=ps[:],
                            func=AF.Relu,
                        )

            # ---------- phase0 + x1(0) (later batches fold into z2) ------
            img_sb = work_pool.tile([P, 4], bf16, tag="img")
            nc.gpsimd.dma_start(
                out=img_sb[:], in_=d_img[0].rearrange("(c p) -> p c", p=P)
            )
            x1_cur = x1_pool.tile([P, NF * V], fp8, tag="x1")
            phase0_half(0)
            p1_cur = build_p1(0, halves=(0,))
            emit_c1(img_sb, p1_cur[0])

            def ilv0(uc2):
                # x1(0) first-half parts ride inside the dense h=1 stream
                if uc2 in (3, 5, 7):
                    emit_x1_part(p1_cur, x1_cur, (uc2 - 3) // 2)

            phase0_half(1, ilv0)
            # second rhs1 half: avt's h=1 columns only exist now
            nc.gpsimd.dma_start(
                out=p1_cur[1][1:4, 1024:2048], in_=avt_bf[0:3, 1024:2048]
            )
            for step in range(3, 8):
                emit_x1_part(p1_cur, x1_cur, step)
            ka_close(ka0)
            # FC weight prefetch, gated behind the critical-path DMAs: a
            # dummy WAW write into fcw1 (sourced from x1) forces Tile to
            # delay the enqueue until x1(0) exists -- by then A2T has fully
            # landed.  The rings share the 16 SDMA engines, so an early fcw
            # enqueue steals HBM bandwidth from the critical path.
            nc.vector.tensor_copy(out=fcw1[0:1, 0:P], in_=x1_cur[0:1, 0:P])
            nc.vector.tensor_copy(out=fcw2[0:1, 0:P], in_=x1_cur[0:1, 0:P])
            nc.vector.tensor_copy(out=fcw3[0:1, 0:P], in_=x1_cur[0:1, 0:P])
            nc.scalar.dma_start(out=fcw1[:], in_=d_fcw1[:])
            nc.scalar.dma_start(out=fcw2[:], in_=d_fcw2[:])
            nc.scalar.dma_start(out=fcw3[:], in_=d_fcw3[:])

            # ---------- per batch: t2 -> z2 (+ interleaved x1/c1/t3) -------
            w34_3d = w34[:].rearrange("p (k j n) -> p k j n", k=2, j=2)
            w56_3d = w56[:].rearrange("p (q j g) -> p q j g", q=2, j=2)
            t3vm_bf = const_pool.tile([P, NV * 16], bf16, tag="vmbf")
            t3vm_f8 = const_pool.tile([P, NV * 16], fp8, tag="vmf8")
            for b in range(BL):
                # t2 = x1 @ W34, vertex-major fp8, DoubleRow over f
                x1_3d = x1_cur[:].rearrange("p (f v) -> p f v", f=NF)
                t2_f8 = t2_pool.tile([P, NV * F], fp8, tag="t2")
                for vc in range(NV):
                    ps = psB.tile([P, F], f32, tag="psB")
                    for fc2 in range(2):
                        nc.tensor.matmul(
                            out=ps[:],
                            lhsT=x1_3d[:, fc2 * 2:fc2 * 2 + 2,
                                       vc * P:(vc + 1) * P],
                            rhs=w34_3d[:, fc2],
                            start=(fc2 == 0),
                            stop=(fc2 == 1),
                            perf_mode=DR,
                        )
                    if vc % 2 == 0:
                        nc.vector.tensor_copy(
                            out=t2_f8[:, vc * F:(vc + 1) * F], in_=ps[:]
                        )
                    else:
                        nc.scalar.copy(
                            out=t2_f8[:, vc * F:(vc + 1) * F], in_=ps[:]
                        )

                # next batch's stream/stationary prep (DMAs overlap z2)
                if b + 1 < BL:
                    img_sb = work_pool.tile([P, 4], bf16, tag="img")
                    nc.gpsimd.dma_start(
                        out=img_sb[:],
                        in_=d_img[b + 1].rearrange("(c p) -> p c", p=P),
                    )
                    p1_nxt = build_p1(b + 1)
                    x1_nxt = x1_pool.tile([P, NF * V], fp8, tag="x1")

                # z2 = A^2 t2 (feature-major out, DoubleRow); x2 = relu.
                # x1(b+1), c1(b+1) and t3(b) matmuls ride inside this stream
                # so the PE never sees a low-activity window.
                t2_3d = t2_f8[:].rearrange("p (u j f) -> p u j f", u=NV2, j=2)
                x2_3d = x2_all[:].rearrange("p (q j v) -> p q j v", q=2, j=2)
                t3t_rows = t3t_all[:].rearrange("(c b) v -> c b v", b=BL)
                # Group order puts the (fc,1) groups early so t3's h=1 half
                # (whose XBAR transpose gates nothing until pair3 part 2)
                # stores+transposes under the last two (fc,0) groups, and
                # the h=0 store chain at the very end is the only exposed
                # latency -- covered by the aggregation's first u-chunks.
                ps_t3 = None
                t3t_b = None
                x1_step = 0
                order = [(0, 0), (0, 1), (1, 0), (1, 1),
                         (2, 1), (3, 1), (2, 0), (3, 0)]

                def t3_store(hh):
                    c0 = hh * 1024
                    nc.vector.tensor_copy(
                        out=t3t_b[:, c0:c0 + 512],
                        in_=ps_t3[0:3, 0:512],
                    )
                    nc.scalar.copy(
                        out=t3t_b[:, c0 + 512:c0 + 1024],
                        in_=ps_t3[0:3, 512:1024],
                    )
                    nc.sync.dma_start(
                        out=t3t_rows[0:3, b, c0:c0 + 1024],
                        in_=t3t_b[:, c0:c0 + 1024],
                    )
                    if b == BL - 1:
                        nc.sync.dma_start(
                            out=t3vm_bf[:, hh * 128:(hh + 1) * 128]
                            .rearrange("p (v g) -> p v g", g=16),
                            in_=t3t_all[0:16, c0:c0 + 1024],
                            transpose=True,
                        )
                        nc.vector.tensor_copy(
                            out=t3vm_f8[:, hh * 128:(hh + 1) * 128],
                            in_=t3vm_bf[:, hh * 128:(hh + 1) * 128],
                        )

                for gi, (fc, nh) in enumerate(order):
                    ps = psA.tile([P, 1024], f32, tag="psA")
                    for uc2 in range(NV2):
                        lhsT = t2_3d[:, uc2, :, fc * P:(fc + 1) * P]
                        rhs3 = a2t[uc2][:].rearrange(
                            "p (j v) -> p j v", j=2
                        )
                        for n2 in range(2):
                            col = nh * 1024 + n2 * 512
                            nc.tensor.matmul(
                                out=ps[:, n2 * 512:(n2 + 1) * 512],
                                lhsT=lhsT,
                                rhs=rhs3[:, :, col:col + 512],
                                start=(uc2 == 0),
                                stop=(uc2 == NV2 - 1 and not has_bias2),
                                perf_mode=DR,
                            )
                    if has_bias2:
                        for n2 in range(2):
                            col = nh * 1024 + n2 * 512
                            nc.tensor.matmul(
                                out=ps[:, n2 * 512:(n2 + 1) * 512],
                                lhsT=biasp2[:, fc * P:(fc + 1) * P],
                                rhs=rho1[:, col:col + 512],
                                start=False,
                                stop=True,
                            )
                    dst = x2_all[:, fc * V + nh * 1024:
                                 fc * V + (nh + 1) * 1024]
                    if fc == 3:
                        # groups gating the t3 store chains: split across
                        # engines so neither blocks the thin t3 copies
                        nc.vector.tensor_scalar_max(
                            out=dst[:, 0:512], in0=ps[:, 0:512],
                            scalar1=0.0,
                        )
                        nc.scalar.activation(
                            out=dst[:, 512:1024], in_=ps[:, 512:1024],
                            func=AF.Relu,
                        )
                    elif nh == 0:
                        nc.vector.tensor_scalar_max(
                            out=dst, in0=ps[:], scalar1=0.0
                        )
                    else:
                        nc.scalar.activation(
                            out=dst, in_=ps[:], func=AF.Relu
                        )

                    # interleave: c1 for the next batch in the first slot,
                    # then the 8 x1 parts front-loaded so their relu
                    # copies land before t2(b+1) needs them
                    if gi == 0 and b + 1 < BL:
                        emit_c1(img_sb, p1_nxt[0])
                    elif b + 1 < BL and x1_step < 8:
                        nparts = 2 if gi in (2, 3, 4) else 1
                        for _ in range(nparts):
                            if x1_step < 8:
                                emit_x1_part(p1_nxt, x1_nxt, x1_step)
                                x1_step += 1
                    # interleave: t3(b) DoubleRow chunks + store chains
                    if gi == 3 or gi == 6:       # q=0 openers (h1, h0)
                        hh = 1 if gi == 3 else 0
                        ps_t3 = psT.tile([16, 1024], f32, tag="psT")
                        for n2 in range(2):
                            nc.tensor.matmul(
                                out=ps_t3[:, n2 * 512:(n2 + 1) * 512],
                                lhsT=w56_3d[:, 0],
                                rhs=x2_3d[:, 0, :,
                                          hh * 1024 + n2 * 512:
                                          hh * 1024 + (n2 + 1) * 512],
                                start=True,
                                stop=False,
                                perf_mode=DR,
                            )
                    elif gi == 5 or gi == 7:     # q=1 closers + stores
                        hh = 1 if gi == 5 else 0
                        for n2 in range(2):
                            nc.tensor.matmul(
                                out=ps_t3[:, n2 * 512:(n2 + 1) * 512],
                                lhsT=w56_3d[:, 1],
                                rhs=x2_3d[:, 1, :,
                                          hh * 1024 + n2 * 512:
                                          hh * 1024 + (n2 + 1) * 512],
                                start=False,
                                stop=True,
                                perf_mode=DR,
                            )
                        if gi == 5:
                            t3t_b = t3b_pool.tile([3, V], bf16, tag="t3t_b")
                        t3_store(hh)

                # any leftover interleave steps for the next batch's x1
                while b + 1 < BL and x1_step < 8:
                    emit_x1_part(p1_nxt, x1_nxt, x1_step)
                    x1_step += 1
                if b + 1 < BL:
                    x1_cur = x1_nxt
                    p1_cur = p1_nxt

            # ---------- pair3 aggregation ----------
            # u-chunks 0-3 need only the early-transposed first t3 half, so
            # they run right after batch 3's tail; chunks 4-7 follow once
            # the second half's transpose (issued mid-tail) lands.  Each
            # output half's relu/transpose/cast is emitted before the other
            # half's remaining matmuls to hide the x3 XBAR latency.
            t3vm_3d = t3vm_f8[:].rearrange("p (u j g) -> p u j g", u=NV2, j=2)
            x3t = const_pool.tile([16, V], bf16, tag="gvec")
            x3vm_bf = const_pool.tile([P, NV * 16], bf16, tag="vmbf2")
            x3f8 = const_pool.tile([P, NV * 16], fp8, tag="vmf82")
            ps_agg = []
            for h in range(2):
                ps_a = psA.tile([16, 1024], f32, tag="psA")
                ps_agg.append(ps_a)

            def agg_part(h, uc2_range, close):
                # u-chunks 4-7 first: their t3 half transposed early (under
                # the last two z2 groups); chunks 0-3's chain hides here
                ps = ps_agg[h]
                for uc2 in uc2_range:
                    rhs3 = a2t[uc2][:].rearrange("p (j v) -> p j v", j=2)
                    for n2 in range(2):
                        col = h * 1024 + n2 * 512
                        nc.tensor.matmul(
                            out=ps[:, n2 * 512:(n2 + 1) * 512],
                            lhsT=t3vm_3d[:, uc2],
                            rhs=rhs3[:, :, col:col + 512],
                            start=(uc2 == 4),
                            stop=(uc2 == 3 and close and not has_bias3),
                            perf_mode=DR,
                        )
                if not close:
                    return
                if has_bias3:
                    for n2 in range(2):
                        col = h * 1024 + n2 * 512
                        nc.tensor.matmul(
                            out=ps[:, n2 * 512:(n2 + 1) * 512],
                            lhsT=biasp3[:],
                            rhs=rho1[:, col:col + 512],
                            start=False,
                            stop=True,
                        )
                nc.vector.tensor_scalar_max(
                    out=x3t[:, h * 1024:(h + 1) * 1024],
                    in0=ps[0:16, :],
                    scalar1=0.0,
                )
                nc.sync.dma_start(
                    out=x3vm_bf[:, h * 128:(h + 1) * 128].rearrange(
                        "p (v g) -> p v g", g=16),
                    in_=x3t[0:16, h * 1024:(h + 1) * 1024],
                    transpose=True,
                )
                nc.vector.tensor_copy(
                    out=x3f8[:, h * 128:(h + 1) * 128],
                    in_=x3vm_bf[:, h * 128:(h + 1) * 128],
                )

            agg_part(0, range(4, NV2), False)
            agg_part(1, range(4, NV2), False)
            agg_part(0, range(4), True)
            agg_part(1, range(4), True)

            # ---------- FC head: resident fp8 weights, DoubleRow, M=BL -----
            # Each hidden layer's transpose runs per n2-half: the first half
            # transposes (XBAR DMA) under the second half's matmuls, and the
            # next layer's first two DoubleRow K-chunks only need the first
            # half, so almost no transpose latency is exposed.
            ka4 = ka_open()

            def h_to_T(ps_h, fcb_dram, emit_group, tg):
                # independent tags: layer N+1's first cast must not wait on
                # layer N's tile release mid-pipeline
                h_sb = hfin_pool.tile([16, FC_H], bf16, tag="hfin" + tg)
                hT_bf = const_pool.tile([P, 8 * 16], bf16, tag="hTbf" + tg)
                hT = const_pool.tile([P, 8 * 16], fp8, tag="hT" + tg)
                if has_fcb:
                    fcb_sb = tail_pool.tile([BL, FC_H], f32, tag="fcb")
                    nc.sync.dma_start(out=fcb_sb[:], in_=fcb_dram[:])
                for n2 in range(2):
                    emit_group(n2)
                    half = slice(n2 * 512, (n2 + 1) * 512)
                    if has_fcb:
                        nc.vector.tensor_add(
                            out=h_sb[0:BL, half], in0=ps_h[:, half],
                            in1=fcb_sb[:, half],
                        )
                    else:
                        # partition-thin copy: split across both engines
                        q0 = n2 * 512
                        nc.vector.tensor_copy(
                            out=h_sb[0:BL, q0:q0 + 256],
                            in_=ps_h[:, q0:q0 + 256],
                        )
                        nc.scalar.copy(
                            out=h_sb[0:BL, q0 + 256:q0 + 512],
                            in_=ps_h[:, q0 + 256:q0 + 512],
                        )
                    nc.sync.dma_start(
                        out=hT_bf[:, n2 * 64:(n2 + 1) * 64].rearrange(
                            "p (k g) -> p k g", g=16),
                        in_=h_sb[0:16, half],
                        transpose=True,
                    )
                    nc.vector.tensor_copy(
                        out=hT[:, n2 * 64:(n2 + 1) * 64],
                        in_=hT_bf[:, n2 * 64:(n2 + 1) * 64],
                    )
                return hT[:].rearrange("p (q j g) -> p q j g", q=4, j=2)

            x3_3d = x3f8[:].rearrange("p (u j g) -> p u j g", u=NV2, j=2)
            fcw1_3d = fcw1[:].rearrange("p (m j n) -> p m j n", m=24, j=2)
            ps_h1 = psA.tile([BL, FC_H], f32, tag="psA")

            def fc1_group(n2):
                # u-major: the first 12 chunks only need the first x3 half,
                # whose transpose completed during the h=1 aggregation
                for i, (u, cc) in enumerate(
                    (u, cc) for u in range(NV2) for cc in range(3)
                ):
                    nc.tensor.matmul(
                        out=ps_h1[:, n2 * 512:(n2 + 1) * 512],
                        lhsT=x3_3d[:, u, :, cc * BL:(cc + 1) * BL],
                        rhs=fcw1_3d[:, cc * NV2 + u, :,
                                    n2 * 512:(n2 + 1) * 512],
                        start=(i == 0),
                        stop=(i == 23),
                        perf_mode=DR,
                    )

            h1T_3d = h_to_T(ps_h1, d_fcb1, fc1_group, "1")
            ka_mm(ka4, 1)

            fcw2_3d = fcw2[:].rearrange("p (q j n) -> p q j n", q=4, j=2)
            ps_h2 = psA.tile([BL, FC_H], f32, tag="psA")

            def fc2_group(n2):
                for q in range(4):
                    nc.tensor.matmul(
                        out=ps_h2[:, n2 * 512:(n2 + 1) * 512],
                        lhsT=h1T_3d[:, q, :, 0:BL],
                        rhs=fcw2_3d[:, q, :, n2 * 512:(n2 + 1) * 512],
                        start=(q == 0),
                        stop=(q == 3),
                        perf_mode=DR,
                    )

            h2T_3d = h_to_T(ps_h2, d_fcb2, fc2_group, "2")
            ka_mm(ka4, 1)
            ka_close(ka4)

            # FC3 + tanh tail (scale by 0.1 and the vertices add happen on
            # the host -- [BL, 1024] tiles are partition-thin and slow on DVE)
            fcw3_3d = fcw3[:].rearrange("p (q j n) -> p q j n", q=4, j=2)
            for ch in range(FLAT // FC_H):
                ps = psA.tile([BL, FC_H], f32, tag="psA")
                for n2 in range(2):
                    for q in range(4):
                        nc.tensor.matmul(
                            out=ps[:, n2 * 512:(n2 + 1) * 512],
                            lhsT=h2T_3d[:, q, :, 0:BL],
                            rhs=fcw3_3d[:, q, :,
                                        ch * FC_H + n2 * 512:
                                        ch * FC_H + (n2 + 1) * 512],
                            start=(q == 0),
                            stop=(q == 3),
                            perf_mode=DR,
                        )
                h3 = tail_pool.tile([BL, FC_H], f32, tag="h3")
                if has_fcb:
                    fcb3_sb = tail_pool.tile([BL, FC_H], f32, tag="fcb")
                    nc.gpsimd.dma_start(
                        out=fcb3_sb[:],
                        in_=d_fcb3[:, ch * FC_H:(ch + 1) * FC_H],
                    )
                    nc.vector.tensor_add(out=h3[:], in0=ps[:], in1=fcb3_sb[:])
                    nc.scalar.activation(out=h3[:], in_=h3[:], func=AF.Tanh)
                else:
                    nc.scalar.activation(out=h3[:], in_=ps[:], func=AF.Tanh)
                nc.sync.dma_start(
                    out=d_out[:, ch * FC_H:(ch + 1) * FC_H], in_=h3[:]
                )

    nc.finalize()
    return nc


def build_in_maps(inputs):
    """Host prep + per-core input maps (exposed for testing)."""
    shared, per_core = _host_prep(inputs)
    key = (shared["HAS_BIAS1"], shared["HAS_BIAS2"], shared["HAS_BIAS3"],
           shared["HAS_FCB"])
    shared_arrays = {k: v for k, v in shared.items() if isinstance(v, np.ndarray)}
    in_maps = []
    for c in range(N_CORES):
        m = dict(shared_arrays)
        m.update(per_core[c])
        in_maps.append(m)
    return key, in_maps


def unpack_out(raw, verts_shard):
    # device returns tanh(h3); deformation scale + vertex add on host
    d = np.asarray(raw, np.float32).reshape(BL, V, 3)
    return np.asarray(verts_shard, np.float32) + 0.1 * d


def kernel(**inputs):
    key, in_maps = build_in_maps(inputs)
    if key not in _CACHE:
        _CACHE[key] = _build_program(*key)
    nc = _CACHE[key]

    from concourse.bass_utils import run_bass_kernel_spmd

    res = run_bass_kernel_spmd(nc, in_maps, list(range(N_CORES)))
    verts = np.asarray(inputs["vertices"], np.float32)
    out = np.empty((B, V, 3), np.float32)
    for c in range(N_CORES):
        out[c * BL:(c + 1) * BL] = unpack_out(
            res.results[c]["OUT"], verts[c * BL:(c + 1) * BL]
        )
    return out
